# revision 39
# baseline (speedup 1.0000x reference)
"""CrossTransformer few-shot attention kernel for 8 Trainium2 NeuronCores.

Data-parallel over the query batch b (32 -> 4 per core); the support set and
1x1-conv weights are replicated on every core. All matmuls run in fp16
(fast weight load + full-rate PE) with fp32 PSUM accumulation; softmax exp is
computed with a constant -12 shift so unnormalized weights stay in fp16 range
(the shift cancels exactly in the normalization).

Self-contained: hardcodes the problem shapes from the spec
(support [25,512,14,14], query [32,512,14,14], W [128,512], 5-way labels).
"""

import sys

import numpy as np

try:
    import concourse.mybir as mybir
except ImportError:
    for _p in ("/opt/trn_rl_repo", "/root/.axon_site/_ro/trn_rl_repo"):
        if _p not in sys.path:
            sys.path.insert(0, _p)
    import concourse.mybir as mybir
import concourse.tile as tile
from concourse import bacc, bass_utils

F32 = mybir.dt.float32
F16 = mybir.dt.float16

N_CORES = 8
B = 32          # query batch
BLOC = B // N_CORES
C = 512         # input channels
HW = 196        # 14*14 positions
D = 128         # dk == dv
NSUP = 25       # support images
KC = 5          # classes
S = NSUP * HW   # 4900 support positions
QB = BLOC * HW  # 784 query columns per core
CC = C // 128   # 4 contraction chunks
NBLK = 5        # support-feature DMA blocks (5 supports each)
BSUP = NSUP // NBLK
HALVES = ((0, 512), (512, 784))  # PSUM-bank-aligned column split (512 f32/bank)
SHIFT = 12.0    # exp(sim - SHIFT): keeps unnormalized softmax in fp16 range

_CACHE: dict = {}


def _chunks(counts):
    """Per-class 128-aligned s-chunks: list of (k, w0, w1, first, last).

    Matmul operands must start at partition 0/32/64, so each class's support
    positions get their own chunk sequence starting at a fresh 128 boundary.
    w0/w1 index into the dense s-axis [0, 4900).
    """
    bs = [0]
    for k in range(KC):
        bs.append(bs[-1] + counts[k] * HW)
    assert bs[-1] == S
    out = []
    for k in range(KC):
        a, b = bs[k], bs[k + 1]
        pos = a
        while pos < b:
            end = min(pos + 128, b)
            out.append((k, pos, end, pos == a, end == b))
            pos = end
    return out


def _build(counts):
    """Build + compile the per-core Bass program for the given class counts."""
    chunks = _chunks(counts)
    NCH = len(chunks)

    nc = bacc.Bacc("TRN2", target_bir_lowering=False, debug=False)

    # host-pretransposed fp16 features: [channels, images, positions]
    sf = nc.dram_tensor("sf", [C, NSUP, HW], F16, kind="ExternalInput")
    qf = nc.dram_tensor("qf", [C, BLOC, HW], F16, kind="ExternalInput")
    w2 = nc.dram_tensor("w2", [2, C, D], F16, kind="ExternalInput")
    ind = nc.dram_tensor("ind", [128, NCH, KC], F16, kind="ExternalInput")
    ident_d = nc.dram_tensor("ident", [128, 128], F16, kind="ExternalInput")
    ones1_d = nc.dram_tensor("ones1", [1, 128], F16, kind="ExternalInput")
    out_pt = nc.dram_tensor("out_pt", [BLOC, KC, D * HW], F32, kind="ExternalOutput")
    out_qv = nc.dram_tensor("out_qv", [BLOC, 1, D * HW], F32, kind="ExternalOutput")

    sf_v = sf.ap().rearrange("(j c) n p -> c j n p", j=CC)      # [128,4,25,196]
    qf_v = qf.ap().rearrange("(j c) b p -> c j b p", j=CC)      # [128,4,4,196]
    w_v = w2.ap().rearrange("w (j c) d -> c w j d", j=CC)       # [128,2,4,128]
    out_pt_v = out_pt.ap().rearrange("b k (c p) -> k c b p", c=D)  # [5,128,4,196]
    out_qv_v = out_qv.ap().rearrange("b one (c p) -> c (b one) p", c=D)  # [128,4,196]

    # class segment geometry (support axis is class-contiguous)
    csum = [0]
    for k in range(KC):
        csum.append(csum[-1] + counts[k])
    cls_chunks = [[] for _ in range(KC)]
    for t, ch in enumerate(chunks):
        cls_chunks[ch[0]].append((t, ch))

    with tile.TileContext(nc) as tc:
        with tc.tile_pool(name="persist", bufs=1) as persist:
            qq_sb = persist.tile([128, QB], F16, tag="qq")
            qv_sb = persist.tile([128, QB], F32, tag="qv")
            expbias = persist.tile([128, 1], F32, tag="expbias")
            nc.vector.memset(expbias, -SHIFT)

            with tc.tile_pool(name="feat", bufs=1) as feat:
                # ---- loads: weights/queries first, per-class support blocks
                # next, late-consumed constants last ----
                # per-c-chunk tiles so the first projection matmul only
                # waits on one weight chunk + one query chunk
                wq_t = feat.tile([128, CC, D], F16, tag="wq")
                nc.sync.dma_start(out=wq_t, in_=w_v[:, 0])
                wq_sb = [wq_t[:, j, :] for j in range(CC)]
                qf_sb = []
                for j in range(CC):
                    t_ = feat.tile(
                        [128, BLOC, HW], F16, tag=f"qf{j}", name=f"qf{j}"
                    )
                    nc.sync.dma_start(out=t_, in_=qf_v[:, j])
                    qf_sb.append(t_.rearrange("c b p -> c (b p)"))
                wv_t = feat.tile([128, CC, D], F16, tag="wv")
                nc.sync.dma_start(out=wv_t, in_=w_v[:, 1])
                wv_sb = [wv_t[:, j, :] for j in range(CC)]
                sfb = {}
                for k in range(KC):
                    if counts[k] == 0:
                        continue
                    t_ = feat.tile(
                        [128, CC, counts[k], HW], F16, tag=f"sfb{k}",
                        name=f"sfb{k}",
                    )
                    nc.sync.dma_start(
                        out=t_, in_=sf_v[:, :, csum[k] : csum[k + 1], :]
                    )
                    sfb[k] = [
                        t_[:, j].rearrange("c n p -> c (n p)") for j in range(CC)
                    ]
                ident = persist.tile([128, 128], F16, tag="ident")
                nc.sync.dma_start(out=ident, in_=ident_d.ap())
                ones1 = persist.tile([1, 128], F16, tag="ones1")
                nc.sync.dma_start(out=ones1, in_=ones1_d.ap())
                ind_sb = persist.tile([128, NCH, KC], F16, tag="ind")
                nc.sync.dma_start(out=ind_sb, in_=ind.ap())

                # ---- query projections ----
                with tc.tile_pool(name="psB", bufs=2, space="PSUM") as psB:
                    for w_, dst in ((wq_sb, qq_sb), (wv_sb, qv_sb)):
                        ps = psB.tile([128, QB], F32, tag="qproj")
                        for n0, n1 in HALVES:
                            for j in range(CC):
                                nc.tensor.matmul(
                                    ps[:, n0:n1],
                                    lhsT=w_[j],
                                    rhs=qf_sb[j][:, n0:n1],
                                    start=(j == 0),
                                    stop=(j == CC - 1),
                                )
                        nc.vector.tensor_copy(dst, ps)
                nc.sync.dma_start(
                    out=out_qv_v,
                    in_=qv_sb.rearrange("c (b p) -> c b p", b=BLOC),
                )

                # ---- per-class support projections + transposes ----
                # class-granular tiles so the attention stream for class k
                # depends only on class k's projections
                sk_t = {}
                svT_t = {}
                sv_t = {}
                with (
                    tc.tile_pool(name="psA", bufs=6, space="PSUM") as psA,
                    tc.tile_pool(name="psT", bufs=2, space="PSUM") as psT,
                ):
                    for k in range(KC):
                        if counts[k] == 0:
                            continue
                        cols = counts[k] * HW
                        sk_k = persist.tile(
                            [128, cols], F16, tag=f"sk{k}", name=f"sk{k}"
                        )
                        sv_k = persist.tile(
                            [128, cols], F16, tag=f"sv{k}", name=f"sv{k}"
                        )
                        nck = len(cls_chunks[k])
                        svT_k = persist.tile(
                            [128, nck, 128], F16, tag=f"svT{k}", name=f"svT{k}"
                        )
                        for w_, dst in ((wq_sb, sk_k), (wv_sb, sv_k)):
                            for o0 in range(0, cols, 512):
                                o1 = min(o0 + 512, cols)
                                ps = psA.tile([128, 512], F32, tag="sproj")
                                for j in range(CC):
                                    nc.tensor.matmul(
                                        ps[:, 0 : o1 - o0],
                                        lhsT=w_[j],
                                        rhs=sfb[k][j][:, o0:o1],
                                        start=(j == 0),
                                        stop=(j == CC - 1),
                                    )
                                nc.vector.tensor_copy(
                                    dst[:, o0:o1], ps[:, 0 : o1 - o0]
                                )
                        base = chunks[cls_chunks[k][0][0]][1]
                        for g in range(0, nck, 4):
                            gn = min(4, nck - g)
                            ps = psT.tile([128, 512], F16, tag="tr")
                            full = True
                            for u in range(gn):
                                _, (_, w0, w1, _, _) = cls_chunks[k][g + u]
                                full = full and (w1 - w0 == 128)
                                nc.tensor.transpose(
                                    out=ps[0 : w1 - w0, u * 128 : (u + 1) * 128],
                                    in_=sv_k[:, w0 - base : w1 - base],
                                    identity=ident,
                                )
                            if full:
                                nc.vector.tensor_copy(
                                    svT_k[:, g : g + gn, :], ps[:, 0 : gn * 128]
                                )
                            else:
                                for u in range(gn):
                                    _, (_, w0, w1, _, _) = cls_chunks[k][g + u]
                                    nc.vector.tensor_copy(
                                        svT_k[0 : w1 - w0, g + u, :],
                                        ps[0 : w1 - w0, u * 128 : (u + 1) * 128],
                                    )
                        sk_t[k] = sk_k
                        sv_t[k] = sv_k
                        svT_t[k] = svT_k

            # ---- attention stream over per-class s-chunks ----
            raws = {}
            with (
                tc.tile_pool(name="exp", bufs=5) as exp_pool,
                tc.tile_pool(name="sim", bufs=2, space="PSUM") as sim_pool,
                tc.tile_pool(name="outp", bufs=1, space="PSUM") as outp_pool,
                tc.tile_pool(name="colsum", bufs=1, space="PSUM") as cs_pool,
            ):
                cs_ps = cs_pool.tile([KC, QB], F32, tag="cs")
                outp = {}
                exps = {}
                nonempty = [k for k in range(KC) if counts[k] > 0]
                first_t = cls_chunks[nonempty[0]][0][0]
                last_t = cls_chunks[nonempty[-1]][-1][0]

                def consume(t, k, w0, w1, first, last, ci, base):
                    rows = w1 - w0
                    exp_t = exps.pop(t)
                    for n0, n1 in HALVES:
                        nc.tensor.matmul(
                            cs_ps[:, n0:n1],
                            lhsT=ind_sb[0:rows, t, :],
                            rhs=exp_t[0:rows, n0:n1],
                            start=(t == first_t),
                            stop=(t == last_t),
                        )
                    if first:
                        outp[k] = outp_pool.tile(
                            [128, QB], F32, tag="outp", name=f"outp{k}"
                        )
                    for n0, n1 in HALVES:
                        nc.tensor.matmul(
                            outp[k][:, n0:n1],
                            lhsT=svT_t[k][0:rows, ci, :],
                            rhs=exp_t[0:rows, n0:n1],
                            start=first,
                            stop=last,
                        )
                    if last:
                        raw = persist.tile(
                            [128, QB], F32, tag=f"raw{k}", name=f"raw{k}"
                        )
                        nc.vector.tensor_copy(raw, outp[k])
                        raws[k] = raw

                # software-pipelined: chunk t's exp consumers are emitted two
                # sim-chunks later so PE never waits on the Scalar exp
                from collections import deque

                pend_q = deque()
                for k in nonempty:
                    base = chunks[cls_chunks[k][0][0]][1]
                    for ci, (t, (_, w0, w1, first, last)) in enumerate(
                        cls_chunks[k]
                    ):
                        rows = w1 - w0
                        ps_sim = sim_pool.tile([128, QB], F32, tag="sim")
                        for n0, n1 in HALVES:
                            nc.tensor.matmul(
                                ps_sim[0:rows, n0:n1],
                                lhsT=sk_t[k][:, w0 - base : w1 - base],
                                rhs=qq_sb[:, n0:n1],
                                start=True,
                                stop=True,
                            )
                        exp_t = exp_pool.tile([128, QB], F16, tag="exp")
                        nc.scalar.activation(
                            out=exp_t[0:rows, :],
                            in_=ps_sim[0:rows, :],
                            func=mybir.ActivationFunctionType.Exp,
                            bias=expbias[0:rows, :],
                        )
                        exps[t] = exp_t
                        pend_q.append((t, k, w0, w1, first, last, ci, base))
                        if len(pend_q) > 2:
                            consume(*pend_q.popleft())
                while pend_q:
                    consume(*pend_q.popleft())

                # reciprocal of the per-class (shifted) denominators.
                # DVE reciprocal cost scales with free-dim, so scatter each
                # 784-wide row across 112 partitions (DMA reshape) and run
                # the reciprocal at FD=35 (~0.4us) instead of FD=784 (~5us).
                # partition-fold the denominator rows so the FD-bound DVE
                # reciprocal runs at FD=35 instead of FD=784
                cs_f32 = persist.tile([KC, QB], F32, tag="cs_f32")
                nc.vector.tensor_copy(cs_f32, cs_ps)
                resh = persist.tile([112, KC, 7], F32, tag="resh")
                if any(c == 0 for c in counts):
                    nc.vector.memset(resh, 1.0)
                for k in range(KC):
                    if counts[k] > 0:
                        nc.sync.dma_start(
                            out=resh[:, k, :], in_=cs_f32[k : k + 1, :]
                        )
                rec_resh = persist.tile([112, KC, 7], F16, tag="recresh")
                with nc.allow_low_precision(reason="denominator broadcast"):
                    nc.vector.reciprocal(rec_resh, resh)

            # ---- normalize + write out ----
            with (
                tc.tile_pool(name="rk", bufs=5) as rk_pool,
                tc.tile_pool(name="outn", bufs=4) as outn_pool,
                tc.tile_pool(name="bc", bufs=4, space="PSUM") as bc_pool,
            ):
                mean_sb = None
                for k in range(KC):
                    outn = outn_pool.tile([128, QB], F32, tag="outn")
                    if counts[k] == 0:
                        # reference semantics: all-masked softmax -> uniform
                        # attention over ALL 4900 support positions
                        if mean_sb is None:
                            rs = persist.tile([128, 1], F32, tag="rs")
                            acc = persist.tile([128, 1], F32, tag="rsacc")
                            started = False
                            for kk in range(KC):
                                if counts[kk] == 0:
                                    continue
                                nc.vector.reduce_sum(
                                    rs if not started else acc,
                                    sv_t[kk],
                                    axis=mybir.AxisListType.X,
                                )
                                if started:
                                    nc.vector.tensor_add(rs, rs, acc)
                                started = True
                            ones_qb = persist.tile([128, QB], F32, tag="onesqb")
                            nc.vector.memset(ones_qb, 1.0)
                            mean_sb = (rs, ones_qb)
                        rs, ones_qb = mean_sb
                        nc.vector.tensor_scalar(
                            out=outn,
                            in0=ones_qb,
                            scalar1=rs,
                            scalar2=1.0 / S,
                            op0=mybir.AluOpType.mult,
                            op1=mybir.AluOpType.mult,
                        )
                    else:
                        # engine ops can't start at partition k>0; DMA the
                        # class's reciprocal row to a partition-0 tile,
                        # broadcast it across partitions via a K=1 matmul,
                        # then multiply
                        rk = rk_pool.tile([1, QB], F16, tag="rk", name=f"rk{k}")
                        nc.sync.dma_start(out=rk, in_=rec_resh[:, k, :])
                        bc = bc_pool.tile([128, QB], F32, tag="bc")
                        for n0, n1 in HALVES:
                            nc.tensor.matmul(
                                bc[:, n0:n1],
                                lhsT=ones1,
                                rhs=rk[:, n0:n1],
                                start=True,
                                stop=True,
                            )
                        nc.vector.tensor_mul(outn, raws[k], bc)
                    nc.sync.dma_start(
                        out=out_pt_v[k], in_=outn.rearrange("c (b p) -> c b p", b=BLOC)
                    )

    nc.compile()
    return nc


def _prepare(support_features, query_features, support_labels, W_qk, W_v):
    sf = np.asarray(support_features, dtype=np.float32).reshape(NSUP, C, HW)
    qf = np.asarray(query_features, dtype=np.float32).reshape(B, C, HW)
    labels = np.asarray(support_labels).astype(np.int64)
    if np.any(np.diff(labels) < 0):  # ensure class-contiguous supports
        order = np.argsort(labels, kind="stable")
        sf = sf[order]
        labels = labels[order]
    counts = tuple(int(x) for x in np.bincount(labels, minlength=KC)[:KC])

    # channels-first fp16 layouts for contiguous per-partition DMA rows
    sf_t = np.ascontiguousarray(sf.transpose(1, 0, 2).astype(np.float16))
    qf_t = qf.transpose(1, 0, 2).astype(np.float16)  # [C, B, HW]
    w2 = np.stack(
        [
            np.asarray(W_qk, dtype=np.float32).T.astype(np.float16),
            np.asarray(W_v, dtype=np.float32).T.astype(np.float16),
        ]
    )  # [2, C, D]

    # indicator [r, t, k]: 1 iff row r of chunk t is a class-k support position
    chunks = _chunks(counts)
    ind = np.zeros((128, len(chunks), KC), np.float16)
    for t, (k, w0, w1, _, _) in enumerate(chunks):
        ind[0 : w1 - w0, t, k] = 1.0
    ind = np.ascontiguousarray(ind)
    return sf_t, qf_t, counts, w2, ind


def _run(support_features, query_features, support_labels, W_qk, W_v, trace=False):
    sf_t, qf_t, counts, w2, ind = _prepare(
        support_features, query_features, support_labels, W_qk, W_v
    )
    if counts not in _CACHE:
        _CACHE[counts] = _build(counts)
    nc = _CACHE[counts]

    ident = np.eye(128, dtype=np.float16)
    ones1 = np.ones((1, 128), np.float16)
    in_maps = []
    for i in range(N_CORES):
        in_maps.append(
            {
                "sf": sf_t,
                "qf": np.ascontiguousarray(qf_t[:, i * BLOC : (i + 1) * BLOC]),
                "w2": w2,
                "ind": ind,
                "ident": ident,
                "ones1": ones1,
            }
        )
    res = bass_utils.run_bass_kernel_spmd(
        nc, in_maps, core_ids=list(range(N_CORES)), trace=trace
    )
    prototypes = np.concatenate(
        [res.results[i]["out_pt"] for i in range(N_CORES)], axis=0
    )
    query_v_flat = np.concatenate(
        [res.results[i]["out_qv"] for i in range(N_CORES)], axis=0
    )
    return (prototypes, query_v_flat), res


def kernel(support_features, query_features, support_labels, W_qk, W_v):
    out, _ = _run(support_features, query_features, support_labels, W_qk, W_v)
    return out


# revision 40
# speedup vs baseline: 1.0130x; 1.0130x over previous
"""CrossTransformer few-shot attention kernel for 8 Trainium2 NeuronCores.

Data-parallel over the query batch b (32 -> 4 per core); the support set and
1x1-conv weights are replicated on every core. All matmuls run in fp16
(fast weight load + full-rate PE) with fp32 PSUM accumulation; softmax exp is
computed with a constant -12 shift so unnormalized weights stay in fp16 range
(the shift cancels exactly in the normalization).

Self-contained: hardcodes the problem shapes from the spec
(support [25,512,14,14], query [32,512,14,14], W [128,512], 5-way labels).
"""

import sys

import numpy as np

try:
    import concourse.mybir as mybir
except ImportError:
    for _p in ("/opt/trn_rl_repo", "/root/.axon_site/_ro/trn_rl_repo"):
        if _p not in sys.path:
            sys.path.insert(0, _p)
    import concourse.mybir as mybir
import concourse.tile as tile
from concourse import bacc, bass_utils

F32 = mybir.dt.float32
F16 = mybir.dt.float16

N_CORES = 8
B = 32          # query batch
BLOC = B // N_CORES
C = 512         # input channels
HW = 196        # 14*14 positions
D = 128         # dk == dv
NSUP = 25       # support images
KC = 5          # classes
S = NSUP * HW   # 4900 support positions
QB = BLOC * HW  # 784 query columns per core
CC = C // 128   # 4 contraction chunks
NBLK = 5        # support-feature DMA blocks (5 supports each)
BSUP = NSUP // NBLK
HALVES = ((0, 512), (512, 784))  # PSUM-bank-aligned column split (512 f32/bank)
SHIFT = 12.0    # exp(sim - SHIFT): keeps unnormalized softmax in fp16 range

_CACHE: dict = {}


def _chunks(counts):
    """Per-class 128-aligned s-chunks: list of (k, w0, w1, first, last).

    Matmul operands must start at partition 0/32/64, so each class's support
    positions get their own chunk sequence starting at a fresh 128 boundary.
    w0/w1 index into the dense s-axis [0, 4900).
    """
    bs = [0]
    for k in range(KC):
        bs.append(bs[-1] + counts[k] * HW)
    assert bs[-1] == S
    out = []
    for k in range(KC):
        a, b = bs[k], bs[k + 1]
        pos = a
        while pos < b:
            end = min(pos + 128, b)
            out.append((k, pos, end, pos == a, end == b))
            pos = end
    return out


def _build(counts):
    """Build + compile the per-core Bass program for the given class counts."""
    chunks = _chunks(counts)
    NCH = len(chunks)

    nc = bacc.Bacc("TRN2", target_bir_lowering=False, debug=False)

    # host-pretransposed fp16 features: [channels, images, positions]
    sf = nc.dram_tensor("sf", [C, NSUP, HW], F16, kind="ExternalInput")
    qf = nc.dram_tensor("qf", [C, BLOC, HW], F16, kind="ExternalInput")
    w2 = nc.dram_tensor("w2", [2, C, D], F16, kind="ExternalInput")
    ind = nc.dram_tensor("ind", [128, NCH, KC], F16, kind="ExternalInput")
    ident_d = nc.dram_tensor("ident", [128, 128], F16, kind="ExternalInput")
    ones1_d = nc.dram_tensor("ones1", [1, 128], F16, kind="ExternalInput")
    out_pt = nc.dram_tensor("out_pt", [BLOC, KC, D * HW], F32, kind="ExternalOutput")
    out_qv = nc.dram_tensor("out_qv", [BLOC, 1, D * HW], F32, kind="ExternalOutput")

    sf_v = sf.ap().rearrange("(j c) n p -> c j n p", j=CC)      # [128,4,25,196]
    qf_v = qf.ap().rearrange("(j c) b p -> c j b p", j=CC)      # [128,4,4,196]
    w_v = w2.ap().rearrange("w (j c) d -> c w j d", j=CC)       # [128,2,4,128]
    out_pt_v = out_pt.ap().rearrange("b k (c p) -> k c b p", c=D)  # [5,128,4,196]
    out_qv_v = out_qv.ap().rearrange("b one (c p) -> c (b one) p", c=D)  # [128,4,196]

    # class segment geometry (support axis is class-contiguous)
    csum = [0]
    for k in range(KC):
        csum.append(csum[-1] + counts[k])
    cls_chunks = [[] for _ in range(KC)]
    for t, ch in enumerate(chunks):
        cls_chunks[ch[0]].append((t, ch))

    with tile.TileContext(nc) as tc:
        with tc.tile_pool(name="persist", bufs=1) as persist:
            qq_sb = persist.tile([128, QB], F16, tag="qq")
            qv_sb = persist.tile([128, QB], F32, tag="qv")
            expbias = persist.tile([128, 1], F32, tag="expbias")
            nc.vector.memset(expbias, -SHIFT)

            with tc.tile_pool(name="feat", bufs=1) as feat:
                # ---- loads: weights/queries first, per-class support blocks
                # next, late-consumed constants last ----
                # per-c-chunk tiles so the first projection matmul only
                # waits on one weight chunk + one query chunk
                wq_t = feat.tile([128, CC, D], F16, tag="wq")
                nc.sync.dma_start(out=wq_t, in_=w_v[:, 0])
                wq_sb = [wq_t[:, j, :] for j in range(CC)]
                qf_sb = []
                for j in range(CC):
                    t_ = feat.tile(
                        [128, BLOC, HW], F16, tag=f"qf{j}", name=f"qf{j}"
                    )
                    nc.sync.dma_start(out=t_, in_=qf_v[:, j])
                    qf_sb.append(t_.rearrange("c b p -> c (b p)"))
                wv_t = feat.tile([128, CC, D], F16, tag="wv")
                nc.sync.dma_start(out=wv_t, in_=w_v[:, 1])
                wv_sb = [wv_t[:, j, :] for j in range(CC)]
                sfb = {}
                for k in range(KC):
                    if counts[k] == 0:
                        continue
                    t_ = feat.tile(
                        [128, CC, counts[k], HW], F16, tag=f"sfb{k}",
                        name=f"sfb{k}",
                    )
                    nc.sync.dma_start(
                        out=t_, in_=sf_v[:, :, csum[k] : csum[k + 1], :]
                    )
                    sfb[k] = [
                        t_[:, j].rearrange("c n p -> c (n p)") for j in range(CC)
                    ]
                ident = persist.tile([128, 128], F16, tag="ident")
                nc.sync.dma_start(out=ident, in_=ident_d.ap())
                ones1 = persist.tile([1, 128], F16, tag="ones1")
                nc.sync.dma_start(out=ones1, in_=ones1_d.ap())
                ind_sb = persist.tile([128, NCH, KC], F16, tag="ind")
                nc.sync.dma_start(out=ind_sb, in_=ind.ap())

                # ---- query projections ----
                with tc.tile_pool(name="psB", bufs=2, space="PSUM") as psB:
                    for w_, dst in ((wq_sb, qq_sb), (wv_sb, qv_sb)):
                        ps = psB.tile([128, QB], F32, tag="qproj")
                        for n0, n1 in HALVES:
                            for j in range(CC):
                                nc.tensor.matmul(
                                    ps[:, n0:n1],
                                    lhsT=w_[j],
                                    rhs=qf_sb[j][:, n0:n1],
                                    start=(j == 0),
                                    stop=(j == CC - 1),
                                )
                        nc.vector.tensor_copy(dst, ps)
                nc.sync.dma_start(
                    out=out_qv_v,
                    in_=qv_sb.rearrange("c (b p) -> c b p", b=BLOC),
                )

                # ---- per-class support projections + transposes ----
                # class-granular tiles so the attention stream for class k
                # depends only on class k's projections
                sk_t = {}
                svT_t = {}
                sv_t = {}
                with (
                    tc.tile_pool(name="psA", bufs=6, space="PSUM") as psA,
                    tc.tile_pool(name="psT", bufs=2, space="PSUM") as psT,
                ):
                    for k in range(KC):
                        if counts[k] == 0:
                            continue
                        cols = counts[k] * HW
                        sk_k = persist.tile(
                            [128, cols], F16, tag=f"sk{k}", name=f"sk{k}"
                        )
                        sv_k = persist.tile(
                            [128, cols], F16, tag=f"sv{k}", name=f"sv{k}"
                        )
                        nck = len(cls_chunks[k])
                        svT_k = persist.tile(
                            [128, nck, 128], F16, tag=f"svT{k}", name=f"svT{k}"
                        )
                        for w_, dst in ((wq_sb, sk_k), (wv_sb, sv_k)):
                            for o0 in range(0, cols, 512):
                                o1 = min(o0 + 512, cols)
                                ps = psA.tile([128, 512], F32, tag="sproj")
                                for j in range(CC):
                                    nc.tensor.matmul(
                                        ps[:, 0 : o1 - o0],
                                        lhsT=w_[j],
                                        rhs=sfb[k][j][:, o0:o1],
                                        start=(j == 0),
                                        stop=(j == CC - 1),
                                    )
                                nc.vector.tensor_copy(
                                    dst[:, o0:o1], ps[:, 0 : o1 - o0]
                                )
                        base = chunks[cls_chunks[k][0][0]][1]
                        for g in range(0, nck, 4):
                            gn = min(4, nck - g)
                            ps = psT.tile([128, 512], F16, tag="tr")
                            full = True
                            for u in range(gn):
                                _, (_, w0, w1, _, _) = cls_chunks[k][g + u]
                                full = full and (w1 - w0 == 128)
                                nc.tensor.transpose(
                                    out=ps[0 : w1 - w0, u * 128 : (u + 1) * 128],
                                    in_=sv_k[:, w0 - base : w1 - base],
                                    identity=ident,
                                )
                            if full:
                                nc.vector.tensor_copy(
                                    svT_k[:, g : g + gn, :], ps[:, 0 : gn * 128]
                                )
                            else:
                                for u in range(gn):
                                    _, (_, w0, w1, _, _) = cls_chunks[k][g + u]
                                    nc.vector.tensor_copy(
                                        svT_k[0 : w1 - w0, g + u, :],
                                        ps[0 : w1 - w0, u * 128 : (u + 1) * 128],
                                    )
                        sk_t[k] = sk_k
                        sv_t[k] = sv_k
                        svT_t[k] = svT_k

            # ---- attention stream over per-class s-chunks ----
            raws = {}
            with (
                tc.tile_pool(name="exp", bufs=5) as exp_pool,
                tc.tile_pool(name="sim", bufs=2, space="PSUM") as sim_pool,
                tc.tile_pool(name="outp", bufs=1, space="PSUM") as outp_pool,
                tc.tile_pool(name="colsum", bufs=1, space="PSUM") as cs_pool,
            ):
                cs_ps = cs_pool.tile([KC, QB], F32, tag="cs")
                outp = {}
                exps = {}
                nonempty = [k for k in range(KC) if counts[k] > 0]
                first_t = cls_chunks[nonempty[0]][0][0]
                last_t = cls_chunks[nonempty[-1]][-1][0]

                def consume(t, k, w0, w1, first, last, ci, base):
                    rows = w1 - w0
                    exp_t = exps.pop(t)
                    for n0, n1 in HALVES:
                        nc.tensor.matmul(
                            cs_ps[:, n0:n1],
                            lhsT=ind_sb[0:rows, t, :],
                            rhs=exp_t[0:rows, n0:n1],
                            start=(t == first_t),
                            stop=(t == last_t),
                        )
                    if first:
                        outp[k] = outp_pool.tile(
                            [128, QB], F32, tag="outp", name=f"outp{k}"
                        )
                    for n0, n1 in HALVES:
                        nc.tensor.matmul(
                            outp[k][:, n0:n1],
                            lhsT=svT_t[k][0:rows, ci, :],
                            rhs=exp_t[0:rows, n0:n1],
                            start=first,
                            stop=last,
                        )
                    if last:
                        raw = persist.tile(
                            [128, QB], F32, tag=f"raw{k}", name=f"raw{k}"
                        )
                        nc.vector.tensor_copy(raw, outp[k])
                        raws[k] = raw

                # software-pipelined: chunk t's exp consumers are emitted two
                # sim-chunks later so PE never waits on the Scalar exp
                from collections import deque

                pend_q = deque()
                for k in nonempty:
                    base = chunks[cls_chunks[k][0][0]][1]
                    for ci, (t, (_, w0, w1, first, last)) in enumerate(
                        cls_chunks[k]
                    ):
                        rows = w1 - w0
                        ps_sim = sim_pool.tile([128, QB], F32, tag="sim")
                        for n0, n1 in HALVES:
                            nc.tensor.matmul(
                                ps_sim[0:rows, n0:n1],
                                lhsT=sk_t[k][:, w0 - base : w1 - base],
                                rhs=qq_sb[:, n0:n1],
                                start=True,
                                stop=True,
                            )
                        exp_t = exp_pool.tile([128, QB], F16, tag="exp")
                        nc.scalar.activation(
                            out=exp_t[0:rows, :],
                            in_=ps_sim[0:rows, :],
                            func=mybir.ActivationFunctionType.Exp,
                            bias=expbias[0:rows, :],
                        )
                        exps[t] = exp_t
                        pend_q.append((t, k, w0, w1, first, last, ci, base))
                        if len(pend_q) > 2:
                            consume(*pend_q.popleft())
                while pend_q:
                    consume(*pend_q.popleft())

                # reciprocal of the per-class (shifted) denominators.
                # DVE reciprocal cost scales with free-dim, so scatter each
                # 784-wide row across 112 partitions (DMA reshape) and run
                # the reciprocal at FD=35 (~0.4us) instead of FD=784 (~5us).
                # partition-fold the denominator rows so the FD-bound DVE
                # reciprocal runs at FD=35 instead of FD=784
                cs_f32 = persist.tile([KC, QB], F32, tag="cs_f32")
                nc.scalar.copy(cs_f32, cs_ps)  # ACT is idle after the last exp
                resh = persist.tile([112, KC, 7], F32, tag="resh")
                if any(c == 0 for c in counts):
                    nc.vector.memset(resh, 1.0)
                for k in range(KC):
                    if counts[k] > 0:
                        nc.sync.dma_start(
                            out=resh[:, k, :], in_=cs_f32[k : k + 1, :]
                        )
                rec_resh = persist.tile([112, KC, 7], F16, tag="recresh")
                with nc.allow_low_precision(reason="denominator broadcast"):
                    nc.vector.reciprocal(rec_resh, resh)

            # ---- normalize + write out ----
            with (
                tc.tile_pool(name="rk", bufs=5) as rk_pool,
                tc.tile_pool(name="outn", bufs=4) as outn_pool,
                tc.tile_pool(name="bc", bufs=4, space="PSUM") as bc_pool,
            ):
                mean_sb = None
                for k in range(KC):
                    outn = outn_pool.tile([128, QB], F32, tag="outn")
                    if counts[k] == 0:
                        # reference semantics: all-masked softmax -> uniform
                        # attention over ALL 4900 support positions
                        if mean_sb is None:
                            rs = persist.tile([128, 1], F32, tag="rs")
                            acc = persist.tile([128, 1], F32, tag="rsacc")
                            started = False
                            for kk in range(KC):
                                if counts[kk] == 0:
                                    continue
                                nc.vector.reduce_sum(
                                    rs if not started else acc,
                                    sv_t[kk],
                                    axis=mybir.AxisListType.X,
                                )
                                if started:
                                    nc.vector.tensor_add(rs, rs, acc)
                                started = True
                            ones_qb = persist.tile([128, QB], F32, tag="onesqb")
                            nc.vector.memset(ones_qb, 1.0)
                            mean_sb = (rs, ones_qb)
                        rs, ones_qb = mean_sb
                        nc.vector.tensor_scalar(
                            out=outn,
                            in0=ones_qb,
                            scalar1=rs,
                            scalar2=1.0 / S,
                            op0=mybir.AluOpType.mult,
                            op1=mybir.AluOpType.mult,
                        )
                    else:
                        # engine ops can't start at partition k>0; DMA the
                        # class's reciprocal row to a partition-0 tile,
                        # broadcast it across partitions via a K=1 matmul,
                        # then multiply
                        rk = rk_pool.tile([1, QB], F16, tag="rk", name=f"rk{k}")
                        nc.sync.dma_start(out=rk, in_=rec_resh[:, k, :])
                        bc = bc_pool.tile([128, QB], F32, tag="bc")
                        for n0, n1 in HALVES:
                            nc.tensor.matmul(
                                bc[:, n0:n1],
                                lhsT=ones1,
                                rhs=rk[:, n0:n1],
                                start=True,
                                stop=True,
                            )
                        nc.vector.tensor_mul(outn, raws[k], bc)
                    nc.sync.dma_start(
                        out=out_pt_v[k], in_=outn.rearrange("c (b p) -> c b p", b=BLOC)
                    )

    nc.compile()
    return nc


def _prepare(support_features, query_features, support_labels, W_qk, W_v):
    sf = np.asarray(support_features, dtype=np.float32).reshape(NSUP, C, HW)
    qf = np.asarray(query_features, dtype=np.float32).reshape(B, C, HW)
    labels = np.asarray(support_labels).astype(np.int64)
    if np.any(np.diff(labels) < 0):  # ensure class-contiguous supports
        order = np.argsort(labels, kind="stable")
        sf = sf[order]
        labels = labels[order]
    counts = tuple(int(x) for x in np.bincount(labels, minlength=KC)[:KC])

    # channels-first fp16 layouts for contiguous per-partition DMA rows
    sf_t = np.ascontiguousarray(sf.transpose(1, 0, 2).astype(np.float16))
    qf_t = qf.transpose(1, 0, 2).astype(np.float16)  # [C, B, HW]
    w2 = np.stack(
        [
            np.asarray(W_qk, dtype=np.float32).T.astype(np.float16),
            np.asarray(W_v, dtype=np.float32).T.astype(np.float16),
        ]
    )  # [2, C, D]

    # indicator [r, t, k]: 1 iff row r of chunk t is a class-k support position
    chunks = _chunks(counts)
    ind = np.zeros((128, len(chunks), KC), np.float16)
    for t, (k, w0, w1, _, _) in enumerate(chunks):
        ind[0 : w1 - w0, t, k] = 1.0
    ind = np.ascontiguousarray(ind)
    return sf_t, qf_t, counts, w2, ind


def _run(support_features, query_features, support_labels, W_qk, W_v, trace=False):
    sf_t, qf_t, counts, w2, ind = _prepare(
        support_features, query_features, support_labels, W_qk, W_v
    )
    if counts not in _CACHE:
        _CACHE[counts] = _build(counts)
    nc = _CACHE[counts]

    ident = np.eye(128, dtype=np.float16)
    ones1 = np.ones((1, 128), np.float16)
    in_maps = []
    for i in range(N_CORES):
        in_maps.append(
            {
                "sf": sf_t,
                "qf": np.ascontiguousarray(qf_t[:, i * BLOC : (i + 1) * BLOC]),
                "w2": w2,
                "ind": ind,
                "ident": ident,
                "ones1": ones1,
            }
        )
    res = bass_utils.run_bass_kernel_spmd(
        nc, in_maps, core_ids=list(range(N_CORES)), trace=trace
    )
    prototypes = np.concatenate(
        [res.results[i]["out_pt"] for i in range(N_CORES)], axis=0
    )
    query_v_flat = np.concatenate(
        [res.results[i]["out_qv"] for i in range(N_CORES)], axis=0
    )
    return (prototypes, query_v_flat), res


def kernel(support_features, query_features, support_labels, W_qk, W_v):
    out, _ = _run(support_features, query_features, support_labels, W_qk, W_v)
    return out
